# revision 1
# baseline (speedup 1.0000x reference)
"""Trainium2 Bass kernel for nn_CrossAttentionFusion (cross-attention + BitLinear FFN).

Sharding: 8 cores = 4 batches x 2 sequence-halves. Each core:
  - owns 1024 query tokens (sem shard, feature-major),
  - computes K/V for its batch's full 2048 tokens from pro (feature-major),
  - runs full attention for its queries + BitLinear FFN, writes its out^T shard.
No collectives needed; host does all layout transposes and the final gather.
"""
import math
import numpy as np
from contextlib import ExitStack

import concourse.bass as bass
import concourse.bass_isa as bass_isa
import concourse.tile as tile
from concourse import bacc, mybir
from concourse.bass_utils import run_bass_kernel_spmd

F32 = mybir.dt.float32
BF16 = mybir.dt.bfloat16
FP8 = mybir.dt.float8e4
AF = mybir.ActivationFunctionType
ALU = mybir.AluOpType

B, S, DS, DP, H = 4, 2048, 1024, 512, 8
DF = 4 * DS
HD = DS // H          # 128
TOK = 1024            # query tokens per core
N_CORES = 8
EPS = 1e-6
C_RND = 12582912.0    # 1.5 * 2**23 : +C-C = round-to-nearest-even
QK_SCALE = 1.0 / math.sqrt(HD)

P = 128
M_SEM = DS // P       # 8
M_PRO = DP // P       # 4
M_FF = DF // P        # 32
NT_Q = TOK // 512     # 2
NT_K = S // P         # 16
MT_V = S // P         # 16


def bcast_free(ap2d, rep):
    """[P, W] AP -> [P, rep, W] AP with step-0 middle dim (free broadcast)."""
    return bass.AP(tensor=ap2d.tensor, offset=ap2d.offset,
                   ap=[ap2d.ap[0], [0, rep], ap2d.ap[1]])


def build_nc(debug_outs=False):
    nc = bacc.Bacc("TRN2", target_bir_lowering=False, debug=False, num_devices=N_CORES)

    semT = nc.dram_tensor("semT", [DS, TOK], F32, kind="ExternalInput").ap()
    proT = nc.dram_tensor("proT", [DP, S], F32, kind="ExternalInput").ap()
    wqT = nc.dram_tensor("wqT", [DS, DS], BF16, kind="ExternalInput").ap()
    wkT = nc.dram_tensor("wkT", [DP, DS], BF16, kind="ExternalInput").ap()
    wvT = nc.dram_tensor("wvT", [DP, DS], BF16, kind="ExternalInput").ap()
    woT = nc.dram_tensor("woT", [DS, DS], BF16, kind="ExternalInput").ap()
    w1T = nc.dram_tensor("w1T", [DS, DF], F32, kind="ExternalInput").ap()
    w2T = nc.dram_tensor("w2T", [DF, DS], F32, kind="ExternalInput").ap()
    w1s = nc.dram_tensor("w1s", [P, DF], F32, kind="ExternalInput").ap()
    w2s = nc.dram_tensor("w2s", [DP, DS], F32, kind="ExternalInput").ap()
    gsem = nc.dram_tensor("gsem", [P, M_SEM], F32, kind="ExternalInput").ap()
    gpro = nc.dram_tensor("gpro", [P, M_PRO], F32, kind="ExternalInput").ap()
    gff = nc.dram_tensor("gff", [P, M_SEM], F32, kind="ExternalInput").ap()
    bq = nc.dram_tensor("bq", [P, M_SEM], F32, kind="ExternalInput").ap()
    bk = nc.dram_tensor("bk", [P, M_SEM], F32, kind="ExternalInput").ap()
    bv = nc.dram_tensor("bv", [P, M_SEM], F32, kind="ExternalInput").ap()
    bo = nc.dram_tensor("bo", [P, M_SEM], F32, kind="ExternalInput").ap()
    alpha = nc.dram_tensor("alpha", [P, M_FF], F32, kind="ExternalInput").ap()
    beta = nc.dram_tensor("beta", [P, M_FF], F32, kind="ExternalInput").ap()
    outT = nc.dram_tensor("outT", [DS, TOK], F32, kind="ExternalOutput").ap()

    dbg = {}
    if debug_outs:
        for name, shape, dt in [
            ("dbg_semn", [DS, TOK], BF16), ("dbg_q", [DS, TOK], BF16),
            ("dbg_k", [DS, S], BF16), ("dbg_v", [S, DS], BF16),
            ("dbg_ctx", [DS, TOK], BF16), ("dbg_semout", [DS, TOK], F32),
            ("dbg_xq", [DS, TOK], BF16), ("dbg_h", [DF, TOK], BF16),
            ("dbg_hq", [DF, TOK], BF16), ("dbg_mw", [1, 2], F32),
        ]:
            dbg[name] = nc.dram_tensor(name, shape, dt, kind="ExternalOutput").ap()

    with tile.TileContext(nc) as tc, ExitStack() as top:
        persist = top.enter_context(tc.tile_pool(name="persist", bufs=1))
        rows = top.enter_context(tc.tile_pool(name="rows", bufs=1))
        ps_mm = top.enter_context(tc.tile_pool(name="ps_mm", bufs=2, space="PSUM"))
        pdram_w = top.enter_context(tc.tile_pool(name="pdram_w", bufs=1,
                                                 space="DRAM"))
        w1q_d = pdram_w.tile([P, M_FF, M_SEM, P], BF16)
        w2q_d = pdram_w.tile([P, M_SEM, M_FF, P], BF16)

        ones = persist.tile([P, 1], BF16)
        nc.vector.memset(ones[:], 1.0)
        ones_row = persist.tile([1, P], BF16)
        nc.vector.memset(ones_row[:], 1.0)
        eps_t = persist.tile([1, 1], F32)
        nc.vector.memset(eps_t[:], EPS)

        gsem_sb = persist.tile([P, M_SEM], F32)
        gpro_sb = persist.tile([P, M_PRO], F32)
        gff_sb = persist.tile([P, M_SEM], F32)
        bq_sb = persist.tile([P, M_SEM], F32)
        bk_sb = persist.tile([P, M_SEM], F32)
        bv_sb = persist.tile([P, M_SEM], F32)
        bo_sb = persist.tile([P, M_SEM], F32)
        alpha_sb = persist.tile([P, M_FF], F32)
        rbeta_sb = persist.tile([P, M_FF], F32)
        for ap_d, t in [(gsem, gsem_sb), (gpro, gpro_sb), (gff, gff_sb),
                        (bq, bq_sb), (bk, bk_sb), (bv, bv_sb), (bo, bo_sb),
                        (alpha, alpha_sb)]:
            nc.sync.dma_start(t[:], ap_d[:])
        beta_t = persist.tile([P, M_FF], F32)
        nc.sync.dma_start(beta_t[:], beta[:])
        nc.vector.tensor_scalar(rbeta_sb[:], beta_t[:], 1e-9, None, ALU.add)
        nc.vector.reciprocal(rbeta_sb[:], rbeta_sb[:])

        semT_r = semT.rearrange("(m p) t -> p m t", p=P)

        def rmsnorm_fm(pool, fetch, nm, T, g_sb, out_bf):
            """feature-major rmsnorm: out_bf[:, m, :] = x_m * g_m * rsqrt(ms+eps)"""
            D = nm * P
            rs_row = pool.tile([1, T], F32, tag="rs_row", bufs=1)
            xs = [fetch(m) for m in range(nm)]
            for ch in range(T // 512):
                pst = ps_mm.tile([P, 512], F32, tag="mm")
                ps = pst[0:1, :]
                for m in range(nm):
                    sq = pool.tile([P, 512], BF16, tag="sq", bufs=3)
                    nc.scalar.activation(sq[:], xs[m][:, ch * 512:(ch + 1) * 512],
                                         AF.Square)
                    nc.tensor.matmul(ps[:], ones[:], sq[:],
                                     start=(m == 0), stop=(m == nm - 1))
                nc.scalar.activation(rs_row[:, ch * 512:(ch + 1) * 512], ps[:],
                                     AF.Ln, bias=eps_t[:], scale=1.0 / D)
            nc.scalar.activation(rs_row[:], rs_row[:], AF.Exp, scale=-0.5)
            rs_bc = pool.tile([P, T], F32, tag="rs_bc", bufs=1)
            nc.gpsimd.partition_broadcast(rs_bc[:], rs_row[:])
            for m in range(nm):
                nc.vector.scalar_tensor_tensor(
                    out=out_bf[:, m, :], in0=xs[m][:],
                    scalar=g_sb[:, m:m + 1], in1=rs_bc[:],
                    op0=ALU.mult, op1=ALU.mult)

        # ================= phase 1: input norms =================
        es_norm = ExitStack()
        pnorm = es_norm.enter_context(tc.tile_pool(name="pnorm", bufs=1))
        semn_sb = pnorm.tile([P, M_SEM, TOK], BF16)
        pron_sb = pnorm.tile([P, M_PRO, S], BF16)

        with tc.tile_pool(name="pin1", bufs=1) as pin1:
            semT_sb = pin1.tile([P, M_SEM, TOK], F32)
            nc.sync.dma_start(semT_sb[:], semT_r)
            rmsnorm_fm(pin1, lambda m: semT_sb[:, m, :], M_SEM, TOK, gsem_sb, semn_sb)

        with tc.tile_pool(name="pin2", bufs=1, side="right") as pin2:
            proT_sb = pin2.tile([P, M_PRO, S], F32)
            nc.sync.dma_start(proT_sb[:], proT.rearrange("(m p) t -> p m t", p=P))
            rmsnorm_fm(pin2, lambda m: proT_sb[:, m, :], M_PRO, S, gpro_sb, pron_sb)

        if debug_outs:
            nc.sync.dma_start(dbg["dbg_semn"].rearrange("(m p) t -> p m t", p=P),
                              semn_sb[:])

        # ================= phase 3: Q/K/V =================
        es_qkv = ExitStack()
        pqkv = es_qkv.enter_context(tc.tile_pool(name="pqkv", bufs=1, side="right"))
        q_sb = pqkv.tile([P, M_SEM, TOK], FP8)
        k_sb = pqkv.tile([P, M_SEM, S], FP8)
        v_sb = pqkv.tile([P, MT_V, DS], BF16)

        with tc.tile_pool(name="pw3", bufs=1) as pw3:
            wq_sb = pw3.tile([P, M_SEM, DS], BF16)
            nc.sync.dma_start(wq_sb[:], wqT.rearrange("(m p) o -> p m o", p=P))
            for m in range(M_SEM):
                for n in range(NT_Q):
                    ps = ps_mm.tile([P, 512], F32, tag="mm")
                    for kk in range(M_SEM):
                        nc.tensor.matmul(ps[:], wq_sb[:, kk, m * P:(m + 1) * P],
                                         semn_sb[:, kk, n * 512:(n + 1) * 512],
                                         start=(kk == 0), stop=(kk == M_SEM - 1))
                    nc.scalar.activation(q_sb[:, m, n * 512:(n + 1) * 512], ps[:],
                                         AF.Identity, bias=bq_sb[:, m:m + 1])

            wk_sb = pw3.tile([P, M_PRO, DS], BF16)
            nc.sync.dma_start(wk_sb[:], wkT.rearrange("(m p) o -> p m o", p=P))
            for m in range(M_SEM):
                for n in range(S // 512):
                    ps = ps_mm.tile([P, 512], F32, tag="mm")
                    for kk in range(M_PRO):
                        nc.tensor.matmul(ps[:], wk_sb[:, kk, m * P:(m + 1) * P],
                                         pron_sb[:, kk, n * 512:(n + 1) * 512],
                                         start=(kk == 0), stop=(kk == M_PRO - 1))
                    nc.scalar.activation(k_sb[:, m, n * 512:(n + 1) * 512], ps[:],
                                         AF.Identity, bias=bk_sb[:, m:m + 1])

            wv_sb = pw3.tile([P, M_PRO, DS], BF16)
            nc.sync.dma_start(wv_sb[:], wvT.rearrange("(m p) o -> p m o", p=P))
            for mt in range(MT_V):
                for n in range(DS // 512):
                    ps = ps_mm.tile([P, 512], F32, tag="mm")
                    for kk in range(M_PRO):
                        nc.tensor.matmul(ps[:], pron_sb[:, kk, mt * P:(mt + 1) * P],
                                         wv_sb[:, kk, n * 512:(n + 1) * 512],
                                         start=(kk == 0), stop=(kk == M_PRO - 1))
                    # bias bv folded in at ctx evac
                    nc.scalar.activation(v_sb[:, mt, n * 512:(n + 1) * 512], ps[:],
                                         AF.Copy)
        es_norm.close()   # semn/pron freed

        # ===== phase 2: mean(|w|) via per-core strips + AllReduce =====
        with tc.tile_pool(name="pwmean", bufs=2) as pwm, \
             tc.tile_pool(name="pdram", bufs=1, space="DRAM") as pdram:
            def strip_sum(ws_ap, nrows, cols, name):
                ntile = nrows // P
                nch = cols // 1024
                mcols = rows.tile([P, ntile * nch], F32, tag=f"mcols_{name}")
                for j in range(ntile):
                    for ci in range(nch):
                        wt = pwm.tile([P, 1024], F32, tag="wmean")
                        nc.sync.dma_start(
                            wt[:], ws_ap[j * P:(j + 1) * P,
                                         ci * 1024:(ci + 1) * 1024])
                        nc.scalar.activation(wt[:], wt[:], AF.Abs,
                                             accum_out=mcols[:, j * nch + ci:
                                                             j * nch + ci + 1])
                msum = rows.tile([P, 1], F32, tag=f"msum_{name}")
                nc.vector.tensor_reduce(msum[:], mcols[:], axis=mybir.AxisListType.X,
                                        op=ALU.add)
                msum_all = rows.tile([P, 1], F32, tag=f"msuma_{name}")
                nc.gpsimd.partition_all_reduce(msum_all[:], msum[:], P,
                                               bass_isa.ReduceOp.add)
                return msum_all

            s1 = strip_sum(w1s, P, DF, "w1")
            s2 = strip_sum(w2s, DP, DS, "w2")
            loc = rows.tile([1, 2], F32, tag="ccloc")
            nc.vector.tensor_copy(loc[:, 0:1], s1[0:1, :])
            nc.vector.tensor_copy(loc[:, 1:2], s2[0:1, :])
            cin = pdram.tile([1, 2], F32)
            cout = pdram.tile([1, 2], F32)
            nc.sync.dma_start(cin[:], loc[:])
            nc.gpsimd.collective_compute(
                "AllReduce", ALU.add,
                replica_groups=[list(range(N_CORES))],
                ins=[cin.opt()], outs=[cout.opt()])
            tot = rows.tile([1, 2], F32, tag="cctot")
            nc.sync.dma_start(tot[:], cout[:])
            mwrow = rows.tile([1, 2], F32, tag="mwrow")
            nc.vector.tensor_scalar(mwrow[:, 0:1], tot[:, 0:1], 1.0 / (DS * DF),
                                    None, ALU.mult)
            nc.vector.tensor_scalar(mwrow[:, 1:2], tot[:, 1:2], 1.0 / (DF * DS),
                                    None, ALU.mult)
            mw_all = rows.tile([P, 2], F32, tag="mwall")
            nc.gpsimd.partition_broadcast(mw_all[:], mwrow[:])
            mw1, mw2 = mw_all[:, 0:1], mw_all[:, 1:2]
            sw_all = rows.tile([P, 2], F32, tag="swall")
            nc.vector.reciprocal(sw_all[:], mw_all[:])
            sw1_bc, sw2_bc = sw_all[:, 0:1], sw_all[:, 1:2]
        if debug_outs:
            nc.sync.dma_start(dbg["dbg_mw"][:], mwrow[:])

        # folded snake scalars: alphap = alpha*mw1 ; rbetap = rbeta/mw1
        alphap = persist.tile([P, M_FF], F32)
        rbetap = persist.tile([P, M_FF], F32)
        nc.vector.tensor_scalar(alphap[:], alpha_sb[:], mw1, None, ALU.mult)
        nc.vector.tensor_scalar(rbetap[:], rbeta_sb[:], sw1_bc, None, ALU.mult)

        # pre-ternarize W1/W2 into DRAM (overlaps QKV/attention)
        with tc.tile_pool(name="ptern", bufs=1, side="right") as ptern:
            w1r_ = w1T.rearrange("(kt p) o -> p kt o", p=P)
            for m in range(M_FF):
                wc = ptern.tile([P, M_SEM, P], F32, tag="w1c", bufs=1)
                nc.sync.dma_start(wc[:], w1r_[:, :, m * P:(m + 1) * P])
                tw = ptern.tile([P, M_SEM * P], F32, tag="terntmp", bufs=1)
                wcf = wc[:].rearrange("p a b -> p (a b)")
                nc.vector.tensor_scalar(tw[:], wcf, sw1_bc, None, ALU.mult)
                nc.vector.tensor_scalar(tw[:], tw[:], 1.49, -1.49, ALU.min,
                                        ALU.max)
                w1q = ptern.tile([P, M_SEM, P], BF16, tag="w1q", bufs=1)
                nc.vector.tensor_scalar(w1q[:].rearrange("p a b -> p (a b)"),
                                        tw[:], C_RND, C_RND, ALU.add,
                                        ALU.subtract)
                nc.sync.dma_start(w1q_d[:, m], w1q[:])
            w2r_ = w2T.rearrange("(kt p) o -> p kt o", p=P)
            for m in range(M_SEM):
                for sub in range(4):
                    wc2 = ptern.tile([P, M_SEM, P], F32, tag="w1c", bufs=1)
                    nc.sync.dma_start(
                        wc2[:], w2r_[:, sub * M_SEM:(sub + 1) * M_SEM,
                                     m * P:(m + 1) * P])
                    tw2 = ptern.tile([P, M_SEM * P], F32, tag="terntmp", bufs=1)
                    wcf2 = wc2[:].rearrange("p a b -> p (a b)")
                    nc.vector.tensor_scalar(tw2[:], wcf2, sw2_bc, None, ALU.mult)
                    nc.vector.tensor_scalar(tw2[:], tw2[:], 1.49, -1.49, ALU.min,
                                            ALU.max)
                    w2q2 = ptern.tile([P, M_SEM, P], BF16, tag="w1q", bufs=1)
                    nc.vector.tensor_scalar(
                        w2q2[:].rearrange("p a b -> p (a b)"), tw2[:], C_RND,
                        C_RND, ALU.add, ALU.subtract)
                    nc.sync.dma_start(
                        w2q_d[:, m, sub * M_SEM:(sub + 1) * M_SEM], w2q2[:])


        if debug_outs:
            nc.sync.dma_start(dbg["dbg_q"].rearrange("(m p) t -> p m t", p=P), q_sb[:])
            nc.sync.dma_start(dbg["dbg_k"].rearrange("(m p) t -> p m t", p=P), k_sb[:])
            nc.sync.dma_start(dbg["dbg_v"].rearrange("(m p) t -> p m t", p=P), v_sb[:])

        # ====== phases 4-9: token-half pipeline (overlap via per-half deps) ======
        es_so = ExitStack()
        psem = es_so.enter_context(tc.tile_pool(name="psem", bufs=1))
        semout_n = [psem.tile([P, M_SEM, 512], F32, tag=f"so{n}", name=f"so{n}")
                    for n in range(NT_Q)]
        es_opr = ExitStack()
        popr = es_opr.enter_context(tc.tile_pool(name="popr", bufs=1))
        wo_sb = popr.tile([P, M_SEM, DS], BF16)
        nc.sync.dma_start(wo_sb[:], woT.rearrange("(m p) o -> p m o", p=P))

        es_ctx = ExitStack()
        pctx = es_ctx.enter_context(tc.tile_pool(name="pctx", bufs=1))
        ctx_n = [pctx.tile([P, M_SEM, 512], BF16, tag=f"ctx{n}", name=f"ctx{n}")
                 for n in range(NT_Q)]

        with tc.tile_pool(name="pattn", bufs=1) as pattn, \
             tc.tile_pool(name="ps_s", bufs=5, space="PSUM") as ps_s:
            for n in range(NT_Q):
                for h in range(H):
                    pt = pattn.tile([P, NT_K, 512], BF16, tag="ptile", bufs=2)
                    for mt in range(NT_K):
                        ps = ps_s.tile([P, 512], F32, tag="sps")
                        nc.tensor.matmul(ps[:], k_sb[:, h, mt * P:(mt + 1) * P],
                                         q_sb[:, h, n * 512:(n + 1) * 512],
                                         start=True, stop=True)
                        nc.scalar.activation(pt[:, mt, :], ps[:], AF.Exp,
                                             scale=QK_SCALE)
                    td = pattn.tile([P, 8, 512], BF16, tag="dentree", bufs=1)
                    ptf = pt[:].rearrange("p a b -> p (a b)")
                    tdf = td[:].rearrange("p a b -> p (a b)")
                    nc.vector.tensor_tensor(tdf[:, 0:4096], ptf[:, 0:4096],
                                            ptf[:, 4096:8192], op=ALU.add)
                    nc.vector.tensor_tensor(tdf[:, 0:2048], tdf[:, 0:2048],
                                            tdf[:, 2048:4096], op=ALU.add)
                    nc.vector.tensor_tensor(tdf[:, 0:1024], tdf[:, 0:1024],
                                            tdf[:, 1024:2048], op=ALU.add)
                    nc.vector.tensor_tensor(tdf[:, 0:512], tdf[:, 0:512],
                                            tdf[:, 512:1024], op=ALU.add)
                    den_all = pattn.tile([P, 512], F32, tag="denall", bufs=2)
                    nc.gpsimd.partition_all_reduce(den_all[:], td[:, 0, :], P,
                                                   bass_isa.ReduceOp.add)
                    rden_bc = pattn.tile([P, 512], F32, tag="rdenbc", bufs=2)
                    nc.vector.reciprocal_approx_fast(rden_bc[:], den_all[:])
                    cps = ps_mm.tile([P, 512], F32, tag="mm")
                    for mt in range(NT_K):
                        nc.tensor.matmul(cps[:], v_sb[:, mt, h * P:(h + 1) * P],
                                         pt[:, mt, :],
                                         start=(mt == 0), stop=(mt == NT_K - 1))
                    tnorm = pattn.tile([P, 512], F32, tag="ctxnorm", bufs=2)
                    nc.vector.tensor_tensor(tnorm[:], cps[:], rden_bc[:],
                                            op=ALU.mult)
                    nc.vector.tensor_scalar(ctx_n[n][:, h, :], tnorm[:],
                                            bv_sb[:, h:h + 1], None, ALU.add)
        es_qkv.close()

        # ---- out-proj ----
        if True:
            for n in range(NT_Q):
                for m in range(M_SEM):
                    semres = popr.tile([P, 512], F32, tag="semres", bufs=2)
                    nc.sync.dma_start(semres[:],
                                      semT_r[:, m, n * 512:(n + 1) * 512])
                    ps = ps_mm.tile([P, 512], F32, tag="mm")
                    for kk in range(M_SEM):
                        nc.tensor.matmul(ps[:],
                                         wo_sb[:, kk, m * P:(m + 1) * P],
                                         ctx_n[n][:, kk, :],
                                         start=(kk == 0),
                                         stop=(kk == M_SEM - 1))
                    t = popr.tile([P, 512], F32, tag="oproj", bufs=3)
                    nc.scalar.activation(t[:], ps[:], AF.Identity,
                                         bias=bo_sb[:, m:m + 1])
                    nc.vector.tensor_tensor(semout_n[n][:, m, :], t[:],
                                            semres[:], op=ALU.add)
        es_ctx.close()
        es_opr.close()

        # ---- FFN tensors (right side) ----
        es_h = ExitStack()
        ph = es_h.enter_context(tc.tile_pool(name="ph", bufs=1, side="right"))
        h_n = [ph.tile([P, M_FF, 512], BF16, tag=f"h{n}", name=f"h{n}") for n in range(NT_Q)]
        mx2_n = [ph.tile([P, 512], BF16, tag=f"mx2{n}", name=f"mx2{n}") for n in range(NT_Q)]
        mn2_n = [ph.tile([P, 512], BF16, tag=f"mn2{n}", name=f"mn2{n}") for n in range(NT_Q)]
        shbc_n = [ph.tile([P, 512], F32, tag=f"shbc{n}", name=f"shbc{n}") for n in range(NT_Q)]
        dq2_n = [ph.tile([P, 512], F32, tag=f"dq2{n}", name=f"dq2{n}") for n in range(NT_Q)]

        es_xq = ExitStack()
        pxq = es_xq.enter_context(tc.tile_pool(name="pxq", bufs=1,
                                               side="right"))
        xq_n = [pxq.tile([P, M_SEM, 512], BF16, tag=f"xq{n}", name=f"xq{n}")
                for n in range(NT_Q)]
        sxbc_n = [pxq.tile([P, 512], F32, tag=f"sxbc{n}", name=f"sxbc{n}")
                  for n in range(NT_Q)]
        rsxbc_n = [pxq.tile([P, 512], F32, tag=f"rsxbc{n}", name=f"rsxbc{n}")
                   for n in range(NT_Q)]

        # ---- whole FFN complex in ONE scratch scope (no pool barriers) ----
        with tc.tile_pool(name="pffs", bufs=1) as pffs:
            def ffnorm_xquant(n):
                xn = pffs.tile([P, M_SEM, 512], BF16, tag="xn", bufs=1)
                rmsnorm_fm(pffs, lambda m: semout_n[n][:, m, :], M_SEM, 512,
                           gff_sb, xn)
                mx = pffs.tile([P, 512], BF16, tag="bt", bufs=4)
                mn = pffs.tile([P, 512], BF16, tag="bt", bufs=4)
                nc.vector.tensor_tensor(mx[:], xn[:, 0, :], xn[:, 1, :],
                                        op=ALU.max)
                nc.vector.tensor_tensor(mn[:], xn[:, 0, :], xn[:, 1, :],
                                        op=ALU.min)
                for m in range(2, M_SEM):
                    nc.vector.tensor_tensor(mx[:], mx[:], xn[:, m, :],
                                            op=ALU.max)
                    nc.vector.tensor_tensor(mn[:], mn[:], xn[:, m, :],
                                            op=ALU.min)
                am = pffs.tile([P, 512], BF16, tag="bt", bufs=4)
                nc.vector.scalar_tensor_tensor(out=am[:], in0=mn[:],
                                               scalar=-1.0, in1=mx[:],
                                               op0=ALU.mult, op1=ALU.max)
                amc = pffs.tile([P, 512], F32, tag="ft", bufs=2)
                nc.gpsimd.partition_all_reduce(amc[:], am[:], P,
                                               bass_isa.ReduceOp.absmax)
                nc.vector.tensor_scalar(amc[:], amc[:], 1e-5, None, ALU.max)
                nc.vector.reciprocal_approx_fast(sxbc_n[n][:], amc[:])
                nc.vector.tensor_scalar(sxbc_n[n][:], sxbc_n[n][:], 127.0,
                                        None, ALU.mult)
                nc.vector.tensor_scalar(rsxbc_n[n][:], amc[:], 1.0 / 127.0,
                                        None, ALU.mult)
                tq = pffs.tile([P, M_SEM, 512], F32, tag="qtw", bufs=1)
                tqf = tq[:].rearrange("p a b -> p (a b)")
                nc.vector.tensor_tensor(tq[:], xn[:],
                                        bcast_free(sxbc_n[n][:], M_SEM),
                                        op=ALU.mult)
                nc.vector.tensor_scalar(tqf[:], tqf[:], C_RND, C_RND, ALU.add,
                                        ALU.subtract)
                nc.vector.tensor_tensor(xq_n[n][:], tq[:],
                                        bcast_free(rsxbc_n[n][:], M_SEM),
                                        op=ALU.mult)

            def ffn1(n):
                for m in range(M_FF):
                    w1q = pffs.tile([P, M_FF, P], BF16, tag="wq", bufs=2)
                    nc.sync.dma_start(w1q[:, :M_SEM, :], w1q_d[:, m])
                    ps = ps_mm.tile([P, 512], F32, tag="mm")
                    for kk in range(M_SEM):
                        nc.tensor.matmul(ps[:], w1q[:, kk, :],
                                         xq_n[n][:, kk, :],
                                         start=(kk == 0),
                                         stop=(kk == M_SEM - 1))
                    sn = pffs.tile([P, 512], BF16, tag="bt", bufs=4)
                    nc.scalar.activation(sn[:], ps[:], AF.Sin,
                                         scale=alphap[:, m:m + 1])
                    sq2 = pffs.tile([P, 512], BF16, tag="bt", bufs=4)
                    nc.scalar.activation(sq2[:], sn[:], AF.Square)
                    nc.vector.scalar_tensor_tensor(
                        out=h_n[n][:, m, :], in0=sq2[:],
                        scalar=rbetap[:, m:m + 1], in1=ps[:],
                        op0=ALU.mult, op1=ALU.add)
                    if m == 0:
                        nc.vector.tensor_copy(mx2_n[n][:], h_n[n][:, 0, :])
                        nc.vector.tensor_copy(mn2_n[n][:], h_n[n][:, 0, :])
                    else:
                        nc.vector.tensor_tensor(mx2_n[n][:], mx2_n[n][:],
                                                h_n[n][:, m, :], op=ALU.max)
                        nc.vector.tensor_tensor(mn2_n[n][:], mn2_n[n][:],
                                                h_n[n][:, m, :], op=ALU.min)

            def hquant(n):
                am2 = pffs.tile([P, 512], BF16, tag="bt", bufs=4)
                nc.vector.scalar_tensor_tensor(out=am2[:], in0=mn2_n[n][:],
                                               scalar=-1.0, in1=mx2_n[n][:],
                                               op0=ALU.mult, op1=ALU.max)
                amc2 = pffs.tile([P, 512], F32, tag="ft", bufs=2)
                nc.gpsimd.partition_all_reduce(amc2[:], am2[:], P,
                                               bass_isa.ReduceOp.absmax)
                nc.vector.tensor_scalar(amc2[:], amc2[:], mw1, 1e-5, ALU.mult,
                                        ALU.max)
                nc.vector.reciprocal_approx_fast(shbc_n[n][:], amc2[:])
                nc.vector.tensor_scalar(shbc_n[n][:], shbc_n[n][:], mw1, 127.0,
                                        ALU.mult, ALU.mult)
                nc.vector.tensor_scalar(dq2_n[n][:], amc2[:], mw2, 1.0 / 127.0,
                                        ALU.mult, ALU.mult)
                for c4 in range(M_FF // M_SEM):
                    tq2 = pffs.tile([P, M_SEM, 512], F32, tag="qtw", bufs=1)
                    tq2f = tq2[:].rearrange("p a b -> p (a b)")
                    nc.vector.tensor_tensor(
                        tq2[:], h_n[n][:, c4 * M_SEM:(c4 + 1) * M_SEM, :],
                        bcast_free(shbc_n[n][:], M_SEM), op=ALU.mult)
                    nc.vector.tensor_scalar(
                        h_n[n][:, c4 * M_SEM:(c4 + 1) * M_SEM, :]
                        .rearrange("p a b -> p (a b)"),
                        tq2f[:], C_RND, C_RND, ALU.add, ALU.subtract)

            def ffn2(n):
                for m in range(M_SEM):
                    w2q = pffs.tile([P, M_FF, P], BF16, tag="wq", bufs=2)
                    nc.sync.dma_start(w2q[:], w2q_d[:, m])
                    ps = ps_mm.tile([P, 512], F32, tag="mm")
                    for kk in range(M_FF):
                        nc.tensor.matmul(ps[:], w2q[:, kk, :], h_n[n][:, kk, :],
                                         start=(kk == 0),
                                         stop=(kk == M_FF - 1))
                    t = pffs.tile([P, 512], F32, tag="qt", bufs=3)
                    nc.vector.tensor_tensor(t[:], ps[:], dq2_n[n][:],
                                            op=ALU.mult)
                    yo = pffs.tile([P, 512], F32, tag="qt", bufs=3)
                    nc.vector.tensor_tensor(yo[:], t[:], semout_n[n][:, m, :],
                                            op=ALU.add)
                    nc.sync.dma_start(outT[m * P:(m + 1) * P,
                                           n * 512:(n + 1) * 512], yo[:])

            ffnorm_xquant(0)
            ffnorm_xquant(1)
            ffn1(0)
            ffn1(1)
            hquant(0)
            hquant(1)
            ffn2(0)
            ffn2(1)
        es_xq.close()
        es_h.close()
        es_so.close()

    nc.compile()
    return nc


_NC_CACHE = {}


def _get_nc(debug_outs=False):
    key = bool(debug_outs)
    if key not in _NC_CACHE:
        _NC_CACHE[key] = build_nc(debug_outs)
    return _NC_CACHE[key]


def make_in_maps(inputs):
    """Host-side shard + layout prep. inputs: dict of full np arrays."""
    import ml_dtypes
    bf = ml_dtypes.bfloat16
    f32 = np.float32
    sem = np.asarray(inputs["sem"], f32)
    pro = np.asarray(inputs["pro"], f32)

    def cols(v, nm):
        return np.ascontiguousarray(np.asarray(v, f32).reshape(nm, P).T)

    common = {
        "gsem": cols(inputs["g_sem"], M_SEM),
        "gpro": cols(inputs["g_pro"], M_PRO),
        "gff": cols(inputs["g_ff"], M_SEM),
        "bq": cols(inputs["bq"], M_SEM),
        "bk": cols(inputs["bk"], M_SEM),
        "bv": cols(inputs["bv"], M_SEM),
        "bo": cols(inputs["bo"], M_SEM),
        "alpha": cols(inputs["alpha"], M_FF),
        "beta": cols(inputs["beta"], M_FF),
        "w1T": np.ascontiguousarray(np.asarray(inputs["W1"], f32).T),
        "w2T": np.ascontiguousarray(np.asarray(inputs["W2"], f32).T),
        "wqT": np.ascontiguousarray(np.asarray(inputs["Wq"], f32).T).astype(bf),
        "wkT": np.ascontiguousarray(np.asarray(inputs["Wk"], f32).T).astype(bf),
        "wvT": np.ascontiguousarray(np.asarray(inputs["Wv"], f32).T).astype(bf),
        "woT": np.ascontiguousarray(np.asarray(inputs["Wo"], f32).T).astype(bf),
    }

    in_maps = []
    for c in range(N_CORES):
        b, half = c // 2, c % 2
        m = dict(common)
        m["semT"] = np.ascontiguousarray(sem[b, half * TOK:(half + 1) * TOK, :].T)
        m["proT"] = np.ascontiguousarray(pro[b].T)
        m["w1s"] = np.ascontiguousarray(common["w1T"][c * P:(c + 1) * P, :])
        m["w2s"] = np.ascontiguousarray(common["w2T"][c * DP:(c + 1) * DP, :])
        in_maps.append(m)
    return in_maps


def assemble_out(results):
    out = np.empty((B, S, DS), np.float32)
    for c in range(N_CORES):
        b, half = c // 2, c % 2
        out[b, half * TOK:(half + 1) * TOK, :] = results[c]["outT"].T
    return out


def kernel(**inputs):
    nc = _get_nc()
    in_maps = make_in_maps(inputs)
    res = run_bass_kernel_spmd(nc, in_maps, core_ids=list(range(N_CORES)))
    return assemble_out(res.results)



# revision 11
# speedup vs baseline: 1.4022x; 1.4022x over previous
"""Trainium2 Bass kernel for nn_CrossAttentionFusion (cross-attention + BitLinear FFN).

Sharding: 8 cores = 4 batches x 2 sequence-halves. Each core:
  - owns 1024 query tokens (sem shard, feature-major),
  - computes K/V for its batch's full 2048 tokens from pro (feature-major),
  - runs full attention for its queries + BitLinear FFN, writes its out^T shard.
No collectives; host does layout prep, weight ternarization and the gather.

v2: fp8 DoubleRow matmuls for all GEMMs except QK^T scores; PE-based softmax
denominator (DR all-ones stationary gives a broadcast denominator directly);
act-quant implemented as a direct fp8 cast with all static scales folded into
the snake/evac constants; bf16 residual trunk; 2-chunk pipeline overlapping
scalar-bound attention with PE-bound FFN2.
"""
import math
import numpy as np
from contextlib import ExitStack

import concourse.bass as bass
import concourse.tile as tile
from concourse import bacc, mybir
from concourse.bass_utils import run_bass_kernel_spmd

F32 = mybir.dt.float32
BF16 = mybir.dt.bfloat16
FP8 = mybir.dt.float8e4
AF = mybir.ActivationFunctionType
ALU = mybir.AluOpType
DR = mybir.MatmulPerfMode.DoubleRow

B, S, DS, DP, H = 4, 2048, 1024, 512, 8
DF = 4 * DS
HD = DS // H          # 128
TOK = 1024            # query tokens per core
N_CORES = 8
EPS = 1e-6
QK_SCALE = 1.0 / math.sqrt(HD)
WSC = 64.0            # host premultiplier on Wq/Wk/Wv/Wo before fp8 cast

P = 128
M_SEM = DS // P       # 8
M_PRO = DP // P       # 4
M_FF = DF // P        # 32
NT_Q = TOK // 512     # 2
NT_K = S // P         # 16


def build_nc(debug_outs=False):
    nc = bacc.Bacc("TRN2", target_bir_lowering=False, debug=False,
                   num_devices=N_CORES)

    semT = nc.dram_tensor("semT", [P, M_SEM, TOK], F32, kind="ExternalInput").ap()
    proT = nc.dram_tensor("proT", [P, M_PRO, S], F32, kind="ExternalInput").ap()
    wq_d = nc.dram_tensor("wq", [P, M_SEM, DS], FP8, kind="ExternalInput").ap()
    wk_d = nc.dram_tensor("wk", [P, M_PRO, DS], FP8, kind="ExternalInput").ap()
    wv_d = nc.dram_tensor("wv", [P, M_PRO, DS], FP8, kind="ExternalInput").ap()
    wo_d = nc.dram_tensor("wo", [P, M_SEM, DS], FP8, kind="ExternalInput").ap()
    w1_d = nc.dram_tensor("w1q", [P, M_SEM, DF], FP8, kind="ExternalInput").ap()
    w2_d = nc.dram_tensor("w2q", [P, M_FF, DS], FP8, kind="ExternalInput").ap()
    gsem = nc.dram_tensor("gsem", [P, M_SEM], F32, kind="ExternalInput").ap()
    gpro = nc.dram_tensor("gpro", [P, M_PRO], F32, kind="ExternalInput").ap()
    gff = nc.dram_tensor("gff", [P, M_SEM], F32, kind="ExternalInput").ap()
    bq_d = nc.dram_tensor("bq", [P, M_SEM], F32, kind="ExternalInput").ap()
    bk_d = nc.dram_tensor("bk", [P, M_SEM], F32, kind="ExternalInput").ap()
    boe_d = nc.dram_tensor("boe", [P, M_SEM], F32, kind="ExternalInput").ap()
    alp_d = nc.dram_tensor("alphap", [P, M_FF], F32, kind="ExternalInput").ap()
    rbp_d = nc.dram_tensor("rbp", [P, M_FF], F32, kind="ExternalInput").ap()
    c2_d = nc.dram_tensor("c2", [P, 1], F32, kind="ExternalInput").ap()
    outT = nc.dram_tensor("outT", [DS, TOK], F32, kind="ExternalOutput").ap()

    dbg = {}
    if debug_outs:
        for name, shape, dt in [
            ("dbg_semn", [P, M_SEM, TOK], FP8), ("dbg_q", [P, M_SEM, TOK], FP8),
            ("dbg_k", [P, M_SEM, S], FP8), ("dbg_v", [P, NT_K, DS], FP8),
            ("dbg_ctx", [P, M_SEM, TOK], FP8),
            ("dbg_semout", [P, M_SEM, TOK], BF16),
            ("dbg_xq", [P, M_SEM, TOK], FP8), ("dbg_hq", [P, M_FF, TOK], FP8),
        ]:
            dbg[name] = nc.dram_tensor(name, shape, dt, kind="ExternalOutput").ap()

    with tile.TileContext(nc) as tc, ExitStack() as top:
        persist = top.enter_context(tc.tile_pool(name="persist", bufs=1))
        ps_sc = top.enter_context(tc.tile_pool(name="ps_sc", bufs=1, space="PSUM"))
        ps_row = top.enter_context(tc.tile_pool(name="ps_row", bufs=1, space="PSUM"))
        ps_ctx = top.enter_context(tc.tile_pool(name="ps_ctx", bufs=1, space="PSUM"))
        ps_mm = top.enter_context(tc.tile_pool(name="ps_mm", bufs=2, space="PSUM"))

        # ---- constants ----
        ones_bf = persist.tile([P, 1], BF16)
        nc.vector.memset(ones_bf[:], 1.0)
        ones_f32 = persist.tile([1, P], F32)
        nc.vector.memset(ones_f32[:], 1.0)
        ones_dr = persist.tile([P, 2, P], FP8)
        nc.vector.memset(ones_dr[:].rearrange("p a b -> p (a b)"), 1.0)
        eps_t = persist.tile([P, 1], F32)
        nc.vector.memset(eps_t[:], EPS)

        gsem_sb = persist.tile([P, M_SEM], F32)
        gpro_sb = persist.tile([P, M_PRO], F32)
        gff_sb = persist.tile([P, M_SEM], F32)
        bq_sb = persist.tile([P, M_SEM], F32)
        bk_sb = persist.tile([P, M_SEM], F32)
        boe_sb = persist.tile([P, M_SEM], F32)
        alp_sb = persist.tile([P, M_FF], F32)
        rbp_sb = persist.tile([P, M_FF], F32)
        c2_sb = persist.tile([P, 1], F32)
        for ap_d, t in [(gsem, gsem_sb), (gpro, gpro_sb), (gff, gff_sb),
                        (bq_d, bq_sb), (bk_d, bk_sb), (boe_d, boe_sb),
                        (alp_d, alp_sb), (rbp_d, rbp_sb), (c2_d, c2_sb)]:
            nc.sync.dma_start(t[:], ap_d[:])

        # ---- big weights, loaded once, resident ----
        wf = top.enter_context(tc.tile_pool(name="wf", bufs=1, side="right"))
        w1_sb = wf.tile([P, M_SEM, DF], FP8)
        w2_sb = wf.tile([P, M_FF, DS], FP8)
        nc.sync.dma_start(w1_sb[:], w1_d[:])
        nc.sync.dma_start(w2_sb[:], w2_d[:])

        def rmsnorm(pool, xs, nm, T, g_sb, out_fp8, Dtot):
            """feature-major rmsnorm -> fp8. xs(m) -> [P, T] f32/bf16 tile."""
            rs_row = pool.tile([1, T], F32, tag="rsrow", bufs=1)
            for ch in range(T // 512):
                ps = ps_row.tile([P, 512], F32, tag="row")
                for m in range(nm):
                    sq = pool.tile([P, 512], BF16, tag="nsq", bufs=3)
                    nc.scalar.activation(sq[:],
                                         xs(m)[:, ch * 512:(ch + 1) * 512],
                                         AF.Square)
                    nc.tensor.matmul(ps[0:1, :], ones_bf[:], sq[:],
                                     start=(m == 0), stop=(m == nm - 1))
                nc.scalar.activation(rs_row[:, ch * 512:(ch + 1) * 512],
                                     ps[0:1, :], AF.Ln, bias=eps_t[0:1, :],
                                     scale=1.0 / Dtot)
            nc.scalar.activation(rs_row[:], rs_row[:], AF.Exp, scale=-0.5)
            for ch in range(T // 512):
                psb = ps_mm.tile([P, 1024], F32, tag="mm")
                nc.tensor.matmul(psb[:, 0:512], ones_f32[:],
                                 rs_row[:, ch * 512:(ch + 1) * 512],
                                 start=True, stop=True)
                for m in range(nm):
                    nc.vector.scalar_tensor_tensor(
                        out=out_fp8[:, m, ch * 512:(ch + 1) * 512],
                        in0=xs(m)[:, ch * 512:(ch + 1) * 512],
                        scalar=g_sb[:, m:m + 1], in1=psb[:, 0:512],
                        op0=ALU.mult, op1=ALU.mult)

        # out-proj weights: allocated early so pool release order stays LIFO
        es_wo = ExitStack()
        pwo = es_wo.enter_context(tc.tile_pool(name="pwo", bufs=1))
        wo_sb = pwo.tile([P, M_SEM, DS], FP8)
        nc.sync.dma_start(wo_sb[:], wo_d[:])

        # ================= P0: input norms =================
        es_nrm = ExitStack()
        pnorm = es_nrm.enter_context(tc.tile_pool(name="pnorm", bufs=1))
        semn = pnorm.tile([P, M_SEM, TOK], FP8)
        pron = pnorm.tile([P, M_PRO, S], FP8)

        with tc.tile_pool(name="pin", bufs=1) as pin:
            semT_sb = pin.tile([P, M_SEM, TOK], F32)
            nc.sync.dma_start(semT_sb[:], semT[:])
            proT_sb = pin.tile([P, M_PRO, S], F32)
            nc.sync.dma_start(proT_sb[:], proT[:])
            rmsnorm(pin, lambda m: semT_sb[:, m, :], M_SEM, TOK, gsem_sb,
                    semn, DS)
            rmsnorm(pin, lambda m: proT_sb[:, m, :], M_PRO, S, gpro_sb,
                    pron, DP)

        if debug_outs:
            nc.sync.dma_start(dbg["dbg_semn"][:], semn[:])

        # ================= P1: Q/K/V projections =================
        es_qkv = ExitStack()
        pqkv = es_qkv.enter_context(tc.tile_pool(name="pqkv", bufs=1,
                                                 side="right"))
        q_sb = pqkv.tile([P, M_SEM, TOK], FP8)
        k_sb = pqkv.tile([P, M_SEM, S], FP8)
        v_sb = pqkv.tile([P, NT_K, DS], FP8)

        with tc.tile_pool(name="pw1", bufs=1) as pw1:
            wq_sb = pw1.tile([P, M_SEM, DS], FP8)
            wk_sb = pw1.tile([P, M_PRO, DS], FP8)
            wv_sb = pw1.tile([P, M_PRO, DS], FP8)
            nc.sync.dma_start(wq_sb[:], wq_d[:])
            nc.sync.dma_start(wk_sb[:], wk_d[:])
            nc.sync.dma_start(wv_sb[:], wv_d[:])

            for m in range(M_SEM):
                ps = ps_mm.tile([P, 1024], F32, tag="mm")
                for n in range(NT_Q):
                    for kp in range(M_SEM // 2):
                        nc.tensor.matmul(
                            ps[:, n * 512:(n + 1) * 512],
                            wq_sb[:, 2 * kp:2 * kp + 2, m * P:(m + 1) * P],
                            semn[:, 2 * kp:2 * kp + 2, n * 512:(n + 1) * 512],
                            start=(kp == 0), stop=(kp == M_SEM // 2 - 1),
                            perf_mode=DR)
                nc.scalar.activation(q_sb[:, m, :], ps[:], AF.Identity,
                                     bias=bq_sb[:, m:m + 1], scale=1.0 / WSC)
            for m in range(M_SEM):
                for chp in range(2):
                    ps = ps_mm.tile([P, 1024], F32, tag="mm")
                    for half in range(2):
                        ch = 2 * chp + half
                        for kp in range(M_PRO // 2):
                            nc.tensor.matmul(
                                ps[:, half * 512:(half + 1) * 512],
                                wk_sb[:, 2 * kp:2 * kp + 2, m * P:(m + 1) * P],
                                pron[:, 2 * kp:2 * kp + 2,
                                     ch * 512:(ch + 1) * 512],
                                start=(kp == 0), stop=(kp == M_PRO // 2 - 1),
                                perf_mode=DR)
                    nc.scalar.activation(
                        k_sb[:, m, chp * 1024:(chp + 1) * 1024], ps[:],
                        AF.Identity, bias=bk_sb[:, m:m + 1], scale=1.0 / WSC)
            for mt in range(NT_K):
                ps = ps_mm.tile([P, 1024], F32, tag="mm")
                for ch in range(2):
                    for kp in range(M_PRO // 2):
                        nc.tensor.matmul(
                            ps[:, ch * 512:(ch + 1) * 512],
                            pron[:, 2 * kp:2 * kp + 2, mt * P:(mt + 1) * P],
                            wv_sb[:, 2 * kp:2 * kp + 2, ch * 512:(ch + 1) * 512],
                            start=(kp == 0), stop=(kp == M_PRO // 2 - 1),
                            perf_mode=DR)
                nc.scalar.activation(v_sb[:, mt, :], ps[:], AF.Copy,
                                     scale=1.0 / WSC)
        es_nrm.close()

        if debug_outs:
            nc.sync.dma_start(dbg["dbg_q"][:], q_sb[:])
            nc.sync.dma_start(dbg["dbg_k"][:], k_sb[:])
            nc.sync.dma_start(dbg["dbg_v"][:], v_sb[:])

        # ============ pipeline state tiles ============
        es_pipe = ExitStack()
        ppipe = es_pipe.enter_context(tc.tile_pool(name="ppipe", bufs=1))
        semout_n = [ppipe.tile([P, M_SEM, 512], BF16, name=f"so{n}")
                    for n in range(NT_Q)]
        xq_n = [ppipe.tile([P, M_SEM, 512], FP8, name=f"xq{n}")
                for n in range(NT_Q)]
        pattn = es_pipe.enter_context(tc.tile_pool(name="pattn", bufs=1))
        pff = es_pipe.enter_context(tc.tile_pool(name="pff", bufs=1,
                                                 side="right"))

        deferred = []

        def drain(k):
            for _ in range(k):
                if deferred:
                    deferred.pop(0)()

        def attn_chunk(n, ctx_t):
            """softmax(q_n K^T) V -> ctx_t [P, M_SEM, 512] fp8 (head-major)."""
            for h in range(H):
                pt = pattn.tile([P, NT_K, 512], FP8, tag="pt", bufs=2)
                for j in range(8):
                    ps = ps_sc.tile([P, 1024], F32, tag="sc")
                    for half in range(2):
                        mt = 2 * j + half
                        nc.tensor.matmul(
                            ps[:, half * 512:(half + 1) * 512],
                            k_sb[:, h, mt * P:(mt + 1) * P],
                            q_sb[:, h, n * 512:(n + 1) * 512],
                            start=True, stop=True)
                    drain(2)
                    nc.scalar.activation(pt[:, 2 * j:2 * j + 2, :], ps[:],
                                         AF.Exp, scale=QK_SCALE)

                def fin(h=h, pt=pt):
                    psd = ps_row.tile([P, 512], F32, tag="row")
                    for j in range(8):
                        nc.tensor.matmul(psd[:], ones_dr[:],
                                         pt[:, 2 * j:2 * j + 2, :],
                                         start=(j == 0), stop=(j == 7),
                                         perf_mode=DR)
                    rden = pattn.tile([P, 512], F32, tag="rden", bufs=2)
                    nc.vector.reciprocal(rden[:], psd[:])
                    psc = ps_ctx.tile([P, 512], F32, tag="ctx")
                    for j in range(8):
                        nc.tensor.matmul(psc[:],
                                         v_sb[:, 2 * j:2 * j + 2,
                                              h * P:(h + 1) * P],
                                         pt[:, 2 * j:2 * j + 2, :],
                                         start=(j == 0), stop=(j == 7),
                                         perf_mode=DR)
                    nc.vector.tensor_tensor(ctx_t[:, h, :], psc[:], rden[:],
                                            op=ALU.mult)
                deferred.append(fin)
            drain(len(deferred))

        def out_proj(n, ctx_t):
            for mg in range(M_SEM // 2):
                semres = ppipe.tile([P, 2, 512], F32, tag="semres", bufs=2)
                nc.sync.dma_start(semres[:],
                                  semT[:, 2 * mg:2 * mg + 2,
                                       n * 512:(n + 1) * 512])
                ps = ps_mm.tile([P, 1024], F32, tag="mm")
                for half in range(2):
                    m = 2 * mg + half
                    for kp in range(M_SEM // 2):
                        nc.tensor.matmul(
                            ps[:, half * 512:(half + 1) * 512],
                            wo_sb[:, 2 * kp:2 * kp + 2, m * P:(m + 1) * P],
                            ctx_t[:, 2 * kp:2 * kp + 2, :],
                            start=(kp == 0), stop=(kp == M_SEM // 2 - 1),
                            perf_mode=DR)
                for half in range(2):
                    m = 2 * mg + half
                    t = pff.tile([P, 512], BF16, tag="oproj", bufs=3)
                    nc.scalar.activation(t[:],
                                         ps[:, half * 512:(half + 1) * 512],
                                         AF.Identity, bias=boe_sb[:, m:m + 1],
                                         scale=1.0 / WSC)
                    nc.vector.tensor_tensor(semout_n[n][:, m, :], t[:],
                                            semres[:, half, :], op=ALU.add)

        def ff_norm(n):
            rmsnorm(pff, lambda m: semout_n[n][:, m, :], M_SEM, 512, gff_sb,
                    xq_n[n], DS)

        def ffn1(n, hq_t):
            for mg in range(M_FF // 2):
                ps = ps_mm.tile([P, 1024], F32, tag="mm")
                for half in range(2):
                    m = 2 * mg + half
                    for kp in range(M_SEM // 2):
                        nc.tensor.matmul(
                            ps[:, half * 512:(half + 1) * 512],
                            w1_sb[:, 2 * kp:2 * kp + 2, m * P:(m + 1) * P],
                            xq_n[n][:, 2 * kp:2 * kp + 2, :],
                            start=(kp == 0), stop=(kp == M_SEM // 2 - 1),
                            perf_mode=DR)
                sn = pff.tile([P, 1024], BF16, tag="sn", bufs=2)
                for half in range(2):
                    m = 2 * mg + half
                    nc.scalar.activation(sn[:, half * 512:(half + 1) * 512],
                                         ps[:, half * 512:(half + 1) * 512],
                                         AF.Sin, scale=alp_sb[:, m:m + 1])
                sq = pff.tile([P, 1024], BF16, tag="sqf", bufs=2)
                nc.vector.tensor_tensor(sq[:], sn[:], sn[:], op=ALU.mult)
                for half in range(2):
                    m = 2 * mg + half
                    nc.vector.scalar_tensor_tensor(
                        out=hq_t[:, m, :],
                        in0=sq[:, half * 512:(half + 1) * 512],
                        scalar=rbp_sb[:, m:m + 1],
                        in1=ps[:, half * 512:(half + 1) * 512],
                        op0=ALU.mult, op1=ALU.add)

        def ffn2_tile(n, mg, hq_t):
            ps = ps_mm.tile([P, 1024], F32, tag="mm")
            for half in range(2):
                m = 2 * mg + half
                for kp in range(M_FF // 2):
                    nc.tensor.matmul(
                        ps[:, half * 512:(half + 1) * 512],
                        w2_sb[:, 2 * kp:2 * kp + 2, m * P:(m + 1) * P],
                        hq_t[:, 2 * kp:2 * kp + 2, :],
                        start=(kp == 0), stop=(kp == M_FF // 2 - 1),
                        perf_mode=DR)
            for half in range(2):
                m = 2 * mg + half
                yo = pff.tile([P, 512], F32, tag="yo", bufs=2)
                nc.vector.scalar_tensor_tensor(
                    out=yo[:], in0=ps[:, half * 512:(half + 1) * 512],
                    scalar=c2_sb[:, 0:1], in1=semout_n[n][:, m, :],
                    op0=ALU.mult, op1=ALU.add)
                nc.sync.dma_start(outT[m * P:(m + 1) * P,
                                       n * 512:(n + 1) * 512], yo[:])

        # ================= P2..P6: pipeline =================
        ctx0 = pattn.tile([P, M_SEM, 512], FP8, tag="ctxt", bufs=1, name="ctx0")
        attn_chunk(0, ctx0)
        out_proj(0, ctx0)
        ff_norm(0)
        if debug_outs:
            nc.sync.dma_start(dbg["dbg_ctx"][:, :, 0:512], ctx0[:])
            nc.sync.dma_start(dbg["dbg_semout"][:, :, 0:512], semout_n[0][:])
            nc.sync.dma_start(dbg["dbg_xq"][:, :, 0:512], xq_n[0][:])

        hq0 = pff.tile([P, M_FF, 512], FP8, tag="hq", bufs=1, name="hq0")
        ffn1(0, hq0)
        if debug_outs:
            nc.sync.dma_start(dbg["dbg_hq"][:, :, 0:512], hq0[:])

        # attention(1) with FFN2(0) zipped into the PE stream
        for mg in range(M_SEM // 2):
            deferred.append(lambda mg=mg: ffn2_tile(0, mg, hq0))
        ctx1 = pattn.tile([P, M_SEM, 512], FP8, tag="ctxt", bufs=1, name="ctx1")
        attn_chunk(1, ctx1)
        out_proj(1, ctx1)
        ff_norm(1)
        if debug_outs:
            nc.sync.dma_start(dbg["dbg_ctx"][:, :, 512:1024], ctx1[:])
            nc.sync.dma_start(dbg["dbg_semout"][:, :, 512:1024], semout_n[1][:])
            nc.sync.dma_start(dbg["dbg_xq"][:, :, 512:1024], xq_n[1][:])

        hq1 = pff.tile([P, M_FF, 512], FP8, tag="hq", bufs=1, name="hq1")
        ffn1(1, hq1)
        if debug_outs:
            nc.sync.dma_start(dbg["dbg_hq"][:, :, 512:1024], hq1[:])
        for mg in range(M_SEM // 2):
            ffn2_tile(1, mg, hq1)

        es_pipe.close()
        es_qkv.close()
        es_wo.close()

    nc.compile()
    return nc


_NC_CACHE = {}


def _get_nc(debug_outs=False):
    key = bool(debug_outs)
    if key not in _NC_CACHE:
        _NC_CACHE[key] = build_nc(debug_outs)
    return _NC_CACHE[key]


def _feat_major(x, nm):
    """[rows, cols] -> [128, nm, cols] with rows = nm*128 split (m p) -> p m."""
    rows, cols = x.shape
    return np.ascontiguousarray(
        x.reshape(nm, P, cols).transpose(1, 0, 2))


def make_in_maps(inputs):
    """Host-side shard + layout prep. inputs: dict of full np arrays."""
    import ml_dtypes
    f8 = ml_dtypes.float8_e4m3fn
    f32 = np.float32
    sem = np.asarray(inputs["sem"], f32)
    pro = np.asarray(inputs["pro"], f32)

    def cols(v, nm):
        return np.ascontiguousarray(np.asarray(v, f32).reshape(nm, P).T)

    W1 = np.asarray(inputs["W1"], f32)
    W2 = np.asarray(inputs["W2"], f32)
    s1 = 1.0 / max(np.abs(W1).mean(dtype=np.float64), 1e-5)
    s2 = 1.0 / max(np.abs(W2).mean(dtype=np.float64), 1e-5)
    w1t = np.clip(np.round(W1 * s1), -1, 1).astype(f32)   # [DF, DS] ternary
    w2t = np.clip(np.round(W2 * s2), -1, 1).astype(f32)   # [DS, DF] ternary

    Wo = np.asarray(inputs["Wo"], f32)
    boe = (np.asarray(inputs["bo"], f32)
           + Wo @ np.asarray(inputs["bv"], f32))

    alpha = np.asarray(inputs["alpha"], f32)
    beta = np.asarray(inputs["beta"], f32)
    alphap = (alpha / s1).astype(f32)
    rbp = (s1 / (beta + 1e-9)).astype(f32)
    c2 = np.full((P, 1), 1.0 / (s1 * s2), f32)

    common = {
        "gsem": cols(inputs["g_sem"], M_SEM),
        "gpro": cols(inputs["g_pro"], M_PRO),
        "gff": cols(inputs["g_ff"], M_SEM),
        "bq": cols(inputs["bq"], M_SEM),
        "bk": cols(inputs["bk"], M_SEM),
        "boe": cols(boe, M_SEM),
        "alphap": cols(alphap, M_FF),
        "rbp": cols(rbp, M_FF),
        "c2": c2,
        "wq": _feat_major(np.asarray(inputs["Wq"], f32).T * WSC, M_SEM).astype(f8),
        "wk": _feat_major(np.asarray(inputs["Wk"], f32).T * WSC, M_PRO).astype(f8),
        "wv": _feat_major(np.asarray(inputs["Wv"], f32).T * WSC, M_PRO).astype(f8),
        "wo": _feat_major(Wo.T * WSC, M_SEM).astype(f8),
        "w1q": _feat_major(np.ascontiguousarray(w1t.T), M_SEM).astype(f8),
        "w2q": _feat_major(np.ascontiguousarray(w2t.T), M_FF).astype(f8),
    }

    in_maps = []
    for c in range(N_CORES):
        b, half = c // 2, c % 2
        m = dict(common)
        m["semT"] = _feat_major(
            np.ascontiguousarray(sem[b, half * TOK:(half + 1) * TOK, :].T),
            M_SEM)
        m["proT"] = _feat_major(np.ascontiguousarray(pro[b].T), M_PRO)
        in_maps.append(m)
    return in_maps


def assemble_out(results):
    out = np.empty((B, S, DS), np.float32)
    for c in range(N_CORES):
        b, half = c // 2, c % 2
        out[b, half * TOK:(half + 1) * TOK, :] = results[c]["outT"].T
    return out


def kernel(**inputs):
    nc = _get_nc()
    in_maps = make_in_maps(inputs)
    res = run_bass_kernel_spmd(nc, in_maps, core_ids=list(range(N_CORES)))
    return assemble_out(res.results)


# revision 30
# speedup vs baseline: 1.9245x; 1.3725x over previous
"""Trainium2 Bass kernel for nn_CrossAttentionFusion (cross-attention + BitLinear FFN).

Sharding: 8 cores = 4 batches x 2 sequence-halves. Each core:
  - owns 1024 query tokens (sem shard, feature-major),
  - computes K/V for its batch's full 2048 tokens from pro (feature-major),
  - runs full attention for its queries + BitLinear FFN, writes its out^T shard.
No collectives; host does layout prep, weight ternarization and the gather.

v2: fp8 DoubleRow matmuls for all GEMMs except QK^T scores; PE-based softmax
denominator (DR all-ones stationary gives a broadcast denominator directly);
act-quant implemented as a direct fp8 cast with all static scales folded into
the snake/evac constants; bf16 residual trunk; 2-chunk pipeline overlapping
scalar-bound attention with PE-bound FFN2.
"""
import math
import numpy as np
from contextlib import ExitStack

import concourse.bass as bass
import concourse.tile as tile
from concourse import bacc, mybir
from concourse.bass_utils import run_bass_kernel_spmd

F32 = mybir.dt.float32
BF16 = mybir.dt.bfloat16
FP8 = mybir.dt.float8e4
AF = mybir.ActivationFunctionType
ALU = mybir.AluOpType
DR = mybir.MatmulPerfMode.DoubleRow

B, S, DS, DP, H = 4, 2048, 1024, 512, 8
DF = 4 * DS
HD = DS // H          # 128
TOK = 1024            # query tokens per core
N_CORES = 8
EPS = 1e-6
QK_SCALE = 1.0 / math.sqrt(HD)
WSC = 64.0            # host premultiplier on Wq/Wk/Wv/Wo before fp8 cast

P = 128
M_SEM = DS // P       # 8
M_PRO = DP // P       # 4
M_FF = DF // P        # 32
NT_Q = TOK // 512     # 2
NT_K = S // P         # 16


def build_nc(debug_outs=False):
    nc = bacc.Bacc("TRN2", target_bir_lowering=False, debug=False,
                   num_devices=N_CORES)

    semT = nc.dram_tensor("semT", [P, M_SEM, TOK], F32, kind="ExternalInput").ap()
    proT = nc.dram_tensor("proT", [P, M_PRO, S], F32, kind="ExternalInput").ap()
    wq_d = nc.dram_tensor("wq", [P, M_SEM, DS], FP8, kind="ExternalInput").ap()
    wk_d = nc.dram_tensor("wk", [P, M_PRO, DS], FP8, kind="ExternalInput").ap()
    wv_d = nc.dram_tensor("wv", [P, M_PRO, DS], FP8, kind="ExternalInput").ap()
    wo_d = nc.dram_tensor("wo", [P, M_SEM, DS], FP8, kind="ExternalInput").ap()
    w1_d = nc.dram_tensor("w1q", [P, M_SEM, DF], FP8, kind="ExternalInput").ap()
    w2_d = nc.dram_tensor("w2q", [P, M_FF, DS], FP8, kind="ExternalInput").ap()
    gsem = nc.dram_tensor("gsem", [P, M_SEM], F32, kind="ExternalInput").ap()
    gpro = nc.dram_tensor("gpro", [P, M_PRO], F32, kind="ExternalInput").ap()
    gff = nc.dram_tensor("gff", [P, M_SEM], F32, kind="ExternalInput").ap()
    bq_d = nc.dram_tensor("bq", [P, M_SEM], F32, kind="ExternalInput").ap()
    bk_d = nc.dram_tensor("bk", [P, M_SEM], F32, kind="ExternalInput").ap()
    boe_d = nc.dram_tensor("boe", [P, M_SEM], F32, kind="ExternalInput").ap()
    alp_d = nc.dram_tensor("alphap", [P, M_FF], F32, kind="ExternalInput").ap()
    rbp_d = nc.dram_tensor("rbp", [P, M_FF], F32, kind="ExternalInput").ap()
    c2_d = nc.dram_tensor("c2", [P, 1], F32, kind="ExternalInput").ap()
    outT = nc.dram_tensor("outT", [DS, TOK], F32, kind="ExternalOutput").ap()

    dbg = {}
    if debug_outs:
        for name, shape, dt in [
            ("dbg_semn", [P, M_SEM, TOK], FP8), ("dbg_q", [P, M_SEM, TOK], FP8),
            ("dbg_k", [P, M_SEM, S], FP8), ("dbg_v", [P, NT_K, DS], FP8),
            ("dbg_ctx", [P, M_SEM, TOK], FP8),
            ("dbg_semout", [P, M_SEM, TOK], BF16),
            ("dbg_xq", [P, M_SEM, TOK], FP8), ("dbg_hq", [P, M_FF, TOK], FP8),
        ]:
            dbg[name] = nc.dram_tensor(name, shape, dt, kind="ExternalOutput").ap()

    with tile.TileContext(nc) as tc, ExitStack() as top:
        persist = top.enter_context(tc.tile_pool(name="persist", bufs=1))
        ps_sc = top.enter_context(tc.tile_pool(name="ps_sc", bufs=2, space="PSUM"))
        ps_row = top.enter_context(tc.tile_pool(name="ps_row", bufs=1, space="PSUM"))
        ps_ctx = top.enter_context(tc.tile_pool(name="ps_ctx", bufs=1, space="PSUM"))
        ps_mm = top.enter_context(tc.tile_pool(name="ps_mm", bufs=1, space="PSUM"))

        # ---- constants ----
        ones_bf = persist.tile([P, 1], BF16)
        nc.vector.memset(ones_bf[:], 1.0)
        ones_f32 = persist.tile([1, P], F32)
        nc.vector.memset(ones_f32[:], 1.0)
        ones_dr = persist.tile([P, 2, P], FP8)
        nc.vector.memset(ones_dr[:].rearrange("p a b -> p (a b)"), 1.0)
        eps_t = persist.tile([P, 1], F32)
        nc.vector.memset(eps_t[:], EPS)

        gsem_sb = persist.tile([P, M_SEM], F32)
        gpro_sb = persist.tile([P, M_PRO], F32)
        gff_sb = persist.tile([P, M_SEM], F32)
        bq_sb = persist.tile([P, M_SEM], F32)
        bk_sb = persist.tile([P, M_SEM], F32)
        boe_sb = persist.tile([P, M_SEM], F32)
        alp_sb = persist.tile([P, M_FF], F32)
        rbp_sb = persist.tile([P, M_FF], F32)
        c2_sb = persist.tile([P, 1], F32)
        for ap_d, t in [(gsem, gsem_sb), (gpro, gpro_sb), (gff, gff_sb),
                        (bq_d, bq_sb), (bk_d, bk_sb), (boe_d, boe_sb),
                        (alp_d, alp_sb), (rbp_d, rbp_sb), (c2_d, c2_sb)]:
            nc.sync.dma_start(t[:], ap_d[:])

        # ---- big weights, resident; DMA'd later on the gpsimd queue ----
        wf = top.enter_context(tc.tile_pool(name="wf", bufs=1, side="right"))
        w1_sb = wf.tile([P, M_SEM, DF], FP8)
        w2_sb = wf.tile([P, M_FF, DS], FP8)

        def rmsnorm(pool, xs, nm, T, g_sb, out_fp8, Dtot):
            """feature-major rmsnorm -> fp8. xs(m) -> [P, T] f32/bf16 tile."""
            rs_row = pool.tile([1, T], F32, tag="rsrow", bufs=1)
            for ch in range(T // 512):
                ps = ps_row.tile([P, 512], F32, tag="row")
                for m in range(nm):
                    sq = pool.tile([P, 512], BF16, tag="nsq", bufs=3)
                    nc.scalar.activation(sq[:],
                                         xs(m)[:, ch * 512:(ch + 1) * 512],
                                         AF.Square)
                    nc.tensor.matmul(ps[0:1, :], ones_bf[:], sq[:],
                                     start=(m == 0), stop=(m == nm - 1))
                nc.scalar.activation(rs_row[:, ch * 512:(ch + 1) * 512],
                                     ps[0:1, :], AF.Ln, bias=eps_t[0:1, :],
                                     scale=1.0 / Dtot)
            nc.scalar.activation(rs_row[:], rs_row[:], AF.Exp, scale=-0.5)
            for ch in range(T // 512):
                psb = ps_mm.tile([P, 1024], F32, tag="mm")
                nc.tensor.matmul(psb[:, 0:512], ones_f32[:],
                                 rs_row[:, ch * 512:(ch + 1) * 512],
                                 start=True, stop=True)
                for m in range(nm):
                    nc.vector.scalar_tensor_tensor(
                        out=out_fp8[:, m, ch * 512:(ch + 1) * 512],
                        in0=xs(m)[:, ch * 512:(ch + 1) * 512],
                        scalar=g_sb[:, m:m + 1], in1=psb[:, 0:512],
                        op0=ALU.mult, op1=ALU.mult)

        # out-proj weights: allocated early so pool release order stays LIFO;
        # DMA'd on the idle gpsimd queue to keep the sync queue for inputs
        es_wo = ExitStack()
        pwo = es_wo.enter_context(tc.tile_pool(name="pwo", bufs=1))
        wo_sb = pwo.tile([P, M_SEM, DS], FP8)
        nc.gpsimd.dma_start(wo_sb[:], wo_d[:])

        # ================= P0: input norms =================
        es_nrm = ExitStack()
        pnorm = es_nrm.enter_context(tc.tile_pool(name="pnorm", bufs=1))
        semn = pnorm.tile([P, M_SEM, TOK], FP8)
        pron = pnorm.tile([P, M_PRO, S], FP8)

        with tc.tile_pool(name="pin", bufs=1) as pin:
            semT_sb = pin.tile([P, M_SEM, TOK], F32)
            nc.sync.dma_start(semT_sb[:], semT[:])
            proT_sb = pin.tile([P, M_PRO, S], F32)
            nc.sync.dma_start(proT_sb[:], proT[:])
            rmsnorm(pin, lambda m: semT_sb[:, m, :], M_SEM, TOK, gsem_sb,
                    semn, DS)
            rmsnorm(pin, lambda m: proT_sb[:, m, :], M_PRO, S, gpro_sb,
                    pron, DP)

        if debug_outs:
            nc.sync.dma_start(dbg["dbg_semn"][:], semn[:])

        # ================= P1: Q/K/V projections =================
        # pff (FFN scratch) sits below pqkv on the right stack so q/k/v can
        # be released before the FFN tail while pff lives on
        es_ff = ExitStack()
        pff = es_ff.enter_context(tc.tile_pool(name="pff", bufs=1,
                                               side="right"))
        es_qkv = ExitStack()
        pqkv = es_qkv.enter_context(tc.tile_pool(name="pqkv", bufs=1,
                                                 side="right"))
        q_sb = pqkv.tile([P, M_SEM, TOK], FP8)
        k_sb = pqkv.tile([P, M_SEM, S], FP8)
        v_sb = pqkv.tile([P, NT_K, DS], FP8)

        with tc.tile_pool(name="pw1", bufs=1) as pw1:
            wq_sb = pw1.tile([P, M_SEM, DS], FP8)
            wk_sb = pw1.tile([P, M_PRO, DS], FP8)
            wv_sb = pw1.tile([P, M_PRO, DS], FP8)
            nc.sync.dma_start(wq_sb[:], wq_d[:])
            nc.sync.dma_start(wk_sb[:], wk_d[:])
            nc.sync.dma_start(wv_sb[:], wv_d[:])

            for m in range(M_SEM):
                ps = ps_sc.tile([P, 1024], F32, tag="sc")
                for n in range(NT_Q):
                    for kp in range(M_SEM // 2):
                        nc.tensor.matmul(
                            ps[:, n * 512:(n + 1) * 512],
                            wq_sb[:, 2 * kp:2 * kp + 2, m * P:(m + 1) * P],
                            semn[:, 2 * kp:2 * kp + 2, n * 512:(n + 1) * 512],
                            start=(kp == 0), stop=(kp == M_SEM // 2 - 1),
                            perf_mode=DR)
                nc.scalar.activation(q_sb[:, m, :], ps[:], AF.Identity,
                                     bias=bq_sb[:, m:m + 1], scale=1.0 / WSC)
            for m in range(M_SEM):
                for chp in range(2):
                    ps = ps_sc.tile([P, 1024], F32, tag="sc")
                    for half in range(2):
                        ch = 2 * chp + half
                        for kp in range(M_PRO // 2):
                            nc.tensor.matmul(
                                ps[:, half * 512:(half + 1) * 512],
                                wk_sb[:, 2 * kp:2 * kp + 2, m * P:(m + 1) * P],
                                pron[:, 2 * kp:2 * kp + 2,
                                     ch * 512:(ch + 1) * 512],
                                start=(kp == 0), stop=(kp == M_PRO // 2 - 1),
                                perf_mode=DR)
                    nc.scalar.activation(
                        k_sb[:, m, chp * 1024:(chp + 1) * 1024], ps[:],
                        AF.Identity, bias=bk_sb[:, m:m + 1], scale=1.0 / WSC)
            for mt in range(NT_K):
                ps = ps_sc.tile([P, 1024], F32, tag="sc")
                for ch in range(2):
                    for kp in range(M_PRO // 2):
                        nc.tensor.matmul(
                            ps[:, ch * 512:(ch + 1) * 512],
                            pron[:, 2 * kp:2 * kp + 2, mt * P:(mt + 1) * P],
                            wv_sb[:, 2 * kp:2 * kp + 2, ch * 512:(ch + 1) * 512],
                            start=(kp == 0), stop=(kp == M_PRO // 2 - 1),
                            perf_mode=DR)
                nc.scalar.activation(v_sb[:, mt, :], ps[:], AF.Copy,
                                     scale=1.0 / WSC)
        es_nrm.close()
        nc.gpsimd.dma_start(w1_sb[:], w1_d[:])
        nc.gpsimd.dma_start(w2_sb[:], w2_d[:])

        if debug_outs:
            nc.sync.dma_start(dbg["dbg_q"][:], q_sb[:])
            nc.sync.dma_start(dbg["dbg_k"][:], k_sb[:])
            nc.sync.dma_start(dbg["dbg_v"][:], v_sb[:])

        # ============ pipeline state tiles ============
        es_pipe = ExitStack()
        ppipe = es_pipe.enter_context(tc.tile_pool(name="ppipe", bufs=1))
        semout_n = [ppipe.tile([P, M_SEM, 512], BF16, name=f"so{n}")
                    for n in range(NT_Q)]
        xq_n = [ppipe.tile([P, M_SEM, 512], FP8, name=f"xq{n}")
                for n in range(NT_Q)]
        es_attn = ExitStack()
        pattn = es_attn.enter_context(tc.tile_pool(name="pattn", bufs=1))

        deferred = []

        def drain(k):
            for _ in range(k):
                if deferred:
                    deferred.pop(0)()

        def attn_chunk(n, ctx_t):
            """softmax(q_n K^T) V -> ctx_t [P, M_SEM, 512] fp8 (head-major)."""
            for h in range(H):
                pt = pattn.tile([P, NT_K, 512], FP8, tag="pt", bufs=2)
                for j in range(8):
                    ps = ps_sc.tile([P, 1024], F32, tag="sc")
                    for half in range(2):
                        mt = 2 * j + half
                        nc.tensor.matmul(
                            ps[:, half * 512:(half + 1) * 512],
                            k_sb[:, h, mt * P:(mt + 1) * P],
                            q_sb[:, h, n * 512:(n + 1) * 512],
                            start=True, stop=True)
                    drain(2)
                    nc.scalar.activation(pt[:, 2 * j:2 * j + 2, :], ps[:],
                                         AF.Exp, scale=QK_SCALE)

                def fin(h=h, pt=pt):
                    psd = ps_row.tile([P, 512], F32, tag="row")
                    for j in range(8):
                        nc.tensor.matmul(psd[:], ones_dr[:],
                                         pt[:, 2 * j:2 * j + 2, :],
                                         start=(j == 0), stop=(j == 7),
                                         perf_mode=DR)
                    rden = pattn.tile([P, 512], F32, tag="rden", bufs=2)
                    nc.vector.reciprocal_approx_fast(rden[:], psd[:])
                    psc = ps_ctx.tile([P, 512], F32, tag="ctx")
                    for j in range(8):
                        nc.tensor.matmul(psc[:],
                                         v_sb[:, 2 * j:2 * j + 2,
                                              h * P:(h + 1) * P],
                                         pt[:, 2 * j:2 * j + 2, :],
                                         start=(j == 0), stop=(j == 7),
                                         perf_mode=DR)
                    nc.vector.tensor_tensor(ctx_t[:, h, :], psc[:], rden[:],
                                            op=ALU.mult)
                deferred.append(fin)
            drain(len(deferred))

        def out_proj(n, ctx_t):
            for mg in range(M_SEM // 2):
                semres = ppipe.tile([P, 2, 512], F32, tag="semres", bufs=2)
                nc.sync.dma_start(semres[:],
                                  semT[:, 2 * mg:2 * mg + 2,
                                       n * 512:(n + 1) * 512])
                ps = ps_mm.tile([P, 1024], F32, tag="mm")
                for half in range(2):
                    m = 2 * mg + half
                    for kp in range(M_SEM // 2):
                        nc.tensor.matmul(
                            ps[:, half * 512:(half + 1) * 512],
                            wo_sb[:, 2 * kp:2 * kp + 2, m * P:(m + 1) * P],
                            ctx_t[:, 2 * kp:2 * kp + 2, :],
                            start=(kp == 0), stop=(kp == M_SEM // 2 - 1),
                            perf_mode=DR)
                for half in range(2):
                    m = 2 * mg + half
                    t = pff.tile([P, 512], BF16, tag="oproj", bufs=3)
                    nc.scalar.activation(t[:],
                                         ps[:, half * 512:(half + 1) * 512],
                                         AF.Identity, bias=boe_sb[:, m:m + 1],
                                         scale=1.0 / WSC)
                    nc.vector.tensor_tensor(semout_n[n][:, m, :], t[:],
                                            semres[:, half, :], op=ALU.add)

        def ff_norm(n):
            rmsnorm(pff, lambda m: semout_n[n][:, m, :], M_SEM, 512, gff_sb,
                    xq_n[n], DS)

        def ffn1(n, hq_t, zip_fn=()):
            zip_fn = list(zip_fn)
            for mg in range(M_FF // 2):
                if mg % 4 == 1 and zip_fn:
                    zip_fn.pop(0)()
                ps = ps_sc.tile([P, 1024], F32, tag="sc")
                for half in range(2):
                    m = 2 * mg + half
                    for kp in range(M_SEM // 2):
                        nc.tensor.matmul(
                            ps[:, half * 512:(half + 1) * 512],
                            w1_sb[:, 2 * kp:2 * kp + 2, m * P:(m + 1) * P],
                            xq_n[n][:, 2 * kp:2 * kp + 2, :],
                            start=(kp == 0), stop=(kp == M_SEM // 2 - 1),
                            perf_mode=DR)
                sn = pff.tile([P, 1024], BF16, tag="sn", bufs=2)
                for half in range(2):
                    m = 2 * mg + half
                    nc.scalar.activation(sn[:, half * 512:(half + 1) * 512],
                                         ps[:, half * 512:(half + 1) * 512],
                                         AF.Sin, scale=alp_sb[:, m:m + 1])
                sq = pff.tile([P, 1024], BF16, tag="sqf", bufs=2)
                nc.vector.tensor_tensor(sq[:], sn[:], sn[:], op=ALU.mult)
                for half in range(2):
                    m = 2 * mg + half
                    nc.vector.scalar_tensor_tensor(
                        out=hq_t[:, m, :],
                        in0=sq[:, half * 512:(half + 1) * 512],
                        scalar=rbp_sb[:, m:m + 1],
                        in1=ps[:, half * 512:(half + 1) * 512],
                        op0=ALU.mult, op1=ALU.add)

        def ffn2_tile(n, mg, hq_t):
            ps = ps_mm.tile([P, 1024], F32, tag="mm")
            for half in range(2):
                m = 2 * mg + half
                for kp in range(M_FF // 2):
                    nc.tensor.matmul(
                        ps[:, half * 512:(half + 1) * 512],
                        w2_sb[:, 2 * kp:2 * kp + 2, m * P:(m + 1) * P],
                        hq_t[:, 2 * kp:2 * kp + 2, :],
                        start=(kp == 0), stop=(kp == M_FF // 2 - 1),
                        perf_mode=DR)
            for half in range(2):
                m = 2 * mg + half
                yo = pff.tile([P, 512], F32, tag="yo", bufs=2)
                nc.vector.scalar_tensor_tensor(
                    out=yo[:], in0=ps[:, half * 512:(half + 1) * 512],
                    scalar=c2_sb[:, 0:1], in1=semout_n[n][:, m, :],
                    op0=ALU.mult, op1=ALU.add)
                nc.sync.dma_start(outT[m * P:(m + 1) * P,
                                       n * 512:(n + 1) * 512], yo[:])

        # ================= P2..P6: pipeline =================
        # attn(0) -> attn(1) zipped with [out_proj(0), ff_norm(0)] ->
        # out_proj(1), ff_norm(1) -> FFN1(0) -> FFN1(1) zipped with FFN2(0)
        # -> FFN2(1).  All Sin activations are grouped at the tail so the
        # scalar engine switches act tables once (exp/ln family -> sin).
        ctx0 = pattn.tile([P, M_SEM, 512], FP8, tag="ctxt", bufs=1, name="ctx0")
        attn_chunk(0, ctx0)

        deferred.append(lambda: out_proj(0, ctx0))
        deferred.append(lambda: ff_norm(0))
        ctx1 = pattn.tile([P, M_SEM, 512], FP8, tag="ctxt", bufs=1, name="ctx1")
        attn_chunk(1, ctx1)
        out_proj(1, ctx1)
        ff_norm(1)
        if debug_outs:
            nc.sync.dma_start(dbg["dbg_ctx"][:, :, 0:512], ctx0[:])
            nc.sync.dma_start(dbg["dbg_semout"][:, :, 0:512], semout_n[0][:])
            nc.sync.dma_start(dbg["dbg_xq"][:, :, 0:512], xq_n[0][:])
            nc.sync.dma_start(dbg["dbg_ctx"][:, :, 512:1024], ctx1[:])
            nc.sync.dma_start(dbg["dbg_semout"][:, :, 512:1024], semout_n[1][:])
            nc.sync.dma_start(dbg["dbg_xq"][:, :, 512:1024], xq_n[1][:])

        es_attn.close()   # pt/rden/ctx freed
        es_qkv.close()    # q/k/v freed before the FFN tail
        es_hq = ExitStack()
        phq = es_hq.enter_context(tc.tile_pool(name="phq", bufs=2,
                                               side="right"))

        hq0 = phq.tile([P, M_FF, 512], FP8, tag="hq", bufs=2, name="hq0")
        ffn1(0, hq0)
        if debug_outs:
            nc.sync.dma_start(dbg["dbg_hq"][:, :, 0:512], hq0[:])

        hq1 = phq.tile([P, M_FF, 512], FP8, tag="hq", bufs=2, name="hq1")
        ffn1(1, hq1, zip_fn=[lambda mg=mg: ffn2_tile(0, mg, hq0)
                             for mg in range(M_SEM // 2)])
        if debug_outs:
            nc.sync.dma_start(dbg["dbg_hq"][:, :, 512:1024], hq1[:])
        for mg in range(M_SEM // 2):
            ffn2_tile(1, mg, hq1)

        es_hq.close()
        es_pipe.close()
        es_ff.close()
        es_wo.close()

    nc.compile()
    return nc


_NC_CACHE = {}


def _get_nc(debug_outs=False):
    key = bool(debug_outs)
    if key not in _NC_CACHE:
        _NC_CACHE[key] = build_nc(debug_outs)
    return _NC_CACHE[key]


def _feat_major(x, nm):
    """[rows, cols] -> [128, nm, cols] with rows = nm*128 split (m p) -> p m."""
    rows, cols = x.shape
    return np.ascontiguousarray(
        x.reshape(nm, P, cols).transpose(1, 0, 2))


def make_in_maps(inputs):
    """Host-side shard + layout prep. inputs: dict of full np arrays."""
    import ml_dtypes
    f8 = ml_dtypes.float8_e4m3fn
    f32 = np.float32
    sem = np.asarray(inputs["sem"], f32)
    pro = np.asarray(inputs["pro"], f32)

    def cols(v, nm):
        return np.ascontiguousarray(np.asarray(v, f32).reshape(nm, P).T)

    W1 = np.asarray(inputs["W1"], f32)
    W2 = np.asarray(inputs["W2"], f32)
    s1 = 1.0 / max(np.abs(W1).mean(dtype=np.float64), 1e-5)
    s2 = 1.0 / max(np.abs(W2).mean(dtype=np.float64), 1e-5)
    w1t = np.clip(np.round(W1 * s1), -1, 1).astype(f32)   # [DF, DS] ternary
    w2t = np.clip(np.round(W2 * s2), -1, 1).astype(f32)   # [DS, DF] ternary

    Wo = np.asarray(inputs["Wo"], f32)
    boe = (np.asarray(inputs["bo"], f32)
           + Wo @ np.asarray(inputs["bv"], f32))

    alpha = np.asarray(inputs["alpha"], f32)
    beta = np.asarray(inputs["beta"], f32)
    alphap = (alpha / s1).astype(f32)
    rbp = (s1 / (beta + 1e-9)).astype(f32)
    c2 = np.full((P, 1), 1.0 / (s1 * s2), f32)

    common = {
        "gsem": cols(inputs["g_sem"], M_SEM),
        "gpro": cols(inputs["g_pro"], M_PRO),
        "gff": cols(inputs["g_ff"], M_SEM),
        "bq": cols(inputs["bq"], M_SEM),
        "bk": cols(inputs["bk"], M_SEM),
        "boe": cols(boe, M_SEM),
        "alphap": cols(alphap, M_FF),
        "rbp": cols(rbp, M_FF),
        "c2": c2,
        "wq": _feat_major(np.asarray(inputs["Wq"], f32).T * WSC, M_SEM).astype(f8),
        "wk": _feat_major(np.asarray(inputs["Wk"], f32).T * WSC, M_PRO).astype(f8),
        "wv": _feat_major(np.asarray(inputs["Wv"], f32).T * WSC, M_PRO).astype(f8),
        "wo": _feat_major(Wo.T * WSC, M_SEM).astype(f8),
        "w1q": _feat_major(np.ascontiguousarray(w1t.T), M_SEM).astype(f8),
        "w2q": _feat_major(np.ascontiguousarray(w2t.T), M_FF).astype(f8),
    }

    in_maps = []
    for c in range(N_CORES):
        b, half = c // 2, c % 2
        m = dict(common)
        m["semT"] = _feat_major(
            np.ascontiguousarray(sem[b, half * TOK:(half + 1) * TOK, :].T),
            M_SEM)
        m["proT"] = _feat_major(np.ascontiguousarray(pro[b].T), M_PRO)
        in_maps.append(m)
    return in_maps


def assemble_out(results):
    out = np.empty((B, S, DS), np.float32)
    for c in range(N_CORES):
        b, half = c // 2, c % 2
        out[b, half * TOK:(half + 1) * TOK, :] = results[c]["outT"].T
    return out


def kernel(**inputs):
    nc = _get_nc()
    in_maps = make_in_maps(inputs)
    res = run_bass_kernel_spmd(nc, in_maps, core_ids=list(range(N_CORES)))
    return assemble_out(res.results)
